# revision 1
# baseline (speedup 1.0000x reference)
"""Trainium2 Bass kernel for nn_JinaPairTraining (dense CE + late-interaction
maxsim CE + KL between the two softmax distributions).

Sharding: data-parallel over the query batch dim Bq (32 rows -> 4 rows on each
of 8 NeuronCores). Every core receives the full pos side, computes its 4 rows
of the dense and maxsim logit matrices, does the row-wise softmax/CE/KL on
device, and returns per-row partials [4, 3] = (-logp_dense, -logp_late, kl).
The host averages the 32 rows (the only "unshard" step).

Key tricks:
  * p_mask is folded on the host by replacing invalid pos tokens with a copy of
    the doc's first valid token -- duplicates never change a max, so no masking
    work on device at all.
  * q_mask is folded into the stationary operand of the sum-over-q matmul
    (a masked one-hot column per (b, q-chunk)), so masking+row-sum+partition
    reduction is a single accumulating matmul chain.
  * matmuls run as float32r (full-rate fp32 on the PE at N>=512).
"""

import os
import sys

import numpy as np

for _p in ("/opt/trn_rl_repo",):
    if _p not in sys.path and os.path.isdir(_p):
        sys.path.insert(0, _p)

import concourse.bacc as bacc
import concourse.bass as bass
import concourse.tile as tile
from concourse import mybir
from concourse.bass_utils import run_bass_kernel_spmd

B, T, D = 32, 256, 128
TAU = 0.02
ITAU = 1.0 / TAU  # 50.0
NCORES = 8
BPC = B // NCORES  # 4 query rows per core
PCOLS = B * T      # 8192 pos token columns
NREG = 4           # p regions of 2048 cols (4 PSUM banks) each
REG = PCOLS // NREG

F32 = mybir.dt.float32
F32R = mybir.dt.float32r
BF16 = mybir.dt.bfloat16
AX = mybir.AxisListType
ALU = mybir.AluOpType
ACT = mybir.ActivationFunctionType


def _build_kernel():
    nc = bacc.Bacc(None, target_bir_lowering=False, debug=False)

    p1_d = nc.dram_tensor("p1T", [D, PCOLS // 2], BF16, kind="ExternalInput")
    pd_d = nc.dram_tensor("pdT", [D, PCOLS // 2], BF16, kind="ExternalInput")
    ident_d = nc.dram_tensor("identity", [128, 128], BF16, kind="ExternalInput")
    qT_d = nc.dram_tensor("qT", [D, 2 * BPC * 128], BF16, kind="ExternalInput")
    qsT_d = nc.dram_tensor("qsT", [D, BPC], F32, kind="ExternalInput")
    psT_d = nc.dram_tensor("psT", [D, B], F32, kind="ExternalInput")
    qoh_d = nc.dram_tensor("qoh", [D, 2 * BPC, BPC], F32, kind="ExternalInput")
    diag_d = nc.dram_tensor("diag_oh", [BPC, B], F32, kind="ExternalInput")
    r50_d = nc.dram_tensor("recip50t", [BPC, 1], F32, kind="ExternalInput")
    out_d = nc.dram_tensor("out", [BPC, 3], F32, kind="ExternalOutput")

    with tile.TileContext(nc) as tc:
        with (
            tc.tile_pool(name="big", bufs=1) as big,
            tc.tile_pool(name="small", bufs=1) as small,
        ):
            # ---- load inputs (p halves in NREG chunks so compute starts early)
            p1T = big.tile([D, NREG, REG // 2], BF16)
            pdT = big.tile([D, NREG, REG // 2], BF16)
            for r in range(NREG):
                nc.sync.dma_start(
                    out=p1T[:, r, :], in_=p1_d[:, r * (REG // 2) : (r + 1) * (REG // 2)]
                )
                nc.sync.dma_start(
                    out=pdT[:, r, :], in_=pd_d[:, r * (REG // 2) : (r + 1) * (REG // 2)]
                )
            ident = small.tile([128, 128], BF16)
            nc.sync.dma_start(out=ident, in_=ident_d[:, :])
            qT = big.tile([D, 2 * BPC * 128], BF16)
            nc.sync.dma_start(out=qT, in_=qT_d[:, :])
            qsT = small.tile([D, BPC], F32)
            nc.sync.dma_start(out=qsT, in_=qsT_d[:, :])
            psT = small.tile([D, B], F32)
            nc.sync.dma_start(out=psT, in_=psT_d[:, :])
            qoh = small.tile([D, 2 * BPC, BPC], F32)
            nc.sync.dma_start(out=qoh, in_=qoh_d[:, :, :])
            diag = small.tile([BPC, B], F32)
            nc.sync.dma_start(out=diag, in_=diag_d[:, :])
            r50 = small.tile([BPC, 1], F32)
            nc.sync.dma_start(out=r50, in_=r50_d[:, :])

            # ---- ACT table warm-up while DMAs stream
            warm_in = small.tile([1, 1], F32)
            nc.vector.memset(warm_in, 1.0)
            warm_out = small.tile([1, 1], F32)
            zeros1 = small.tile([BPC, 1], F32)
            nc.vector.memset(zeros1, 0.0)
            nc.scalar.activation(warm_out, warm_in, ACT.Exp, bias=zeros1[0:1, :])
            nc.scalar.activation(warm_out, warm_out, ACT.Ln, bias=zeros1[0:1, :])

            # mx[q, j, c]: per (b, q-chunk) j, per pos-doc c, the masked max
            # over that doc's 256 token sims.
            mx = small.tile([128, 2 * BPC, B], F32)

            # ---- main streamed phase.  Pair-max is folded into PE+ACT via
            # max(s0, s1) = s1 + relu(s0 - s1): PE computes Q@(P0-P1) and
            # Q@P1, ACT applies relu, an identity matmul accumulates it back
            # into the Q@P1 PSUM tile.  DVE then reduces 128 (not 256)
            # values per doc -- halving the 1x PSUM-read bottleneck.
            with (
                tc.tile_pool(name="psum_big", bufs=2, space="PSUM") as pb,
                tc.tile_pool(name="relu_pool", bufs=2) as rp,
            ):
                HREG = REG // 2  # 1024 cols per region after pair fold
                for r in range(NREG):          # p region: 8 docs x 128 pairs
                    for j in range(2 * BPC):   # (b, q-chunk)
                        ps_d = pb.tile([128, HREG], F32, name="ps_d")
                        ps_m = pb.tile([128, HREG], F32, name="ps_m")
                        for k in range(HREG // 512):
                            nc.tensor.matmul(
                                ps_d[:, k * 512 : (k + 1) * 512],
                                qT[:, j * 128 : (j + 1) * 128],
                                pdT[:, r, k * 512 : (k + 1) * 512],
                                start=True,
                                stop=True,
                            )
                        for k in range(HREG // 512):
                            nc.tensor.matmul(
                                ps_m[:, k * 512 : (k + 1) * 512],
                                qT[:, j * 128 : (j + 1) * 128],
                                p1T[:, r, k * 512 : (k + 1) * 512],
                                start=True,
                                stop=False,
                            )
                        relu_sb = rp.tile([128, HREG], BF16, name="relu_sb")
                        nc.scalar.activation(relu_sb, ps_d, ACT.Relu)
                        for k in range(HREG // 512):
                            nc.tensor.matmul(
                                ps_m[:, k * 512 : (k + 1) * 512],
                                ident,
                                relu_sb[:, k * 512 : (k + 1) * 512],
                                start=False,
                                stop=True,
                            )
                        nc.vector.reduce_max(
                            out=mx[:, j, r * (REG // T) : (r + 1) * (REG // T)],
                            in_=ps_m.rearrange("p (g s) -> p g s", s=T // 2),
                            axis=AX.X,
                        )

            # ---- tail: S_late rows, dense rows, softmax/CE/KL
            with tc.tile_pool(name="psum_small", bufs=1, space="PSUM") as pss:
                s_ps = pss.tile([BPC, B], F32)
                for j in range(2 * BPC):
                    nc.tensor.matmul(
                        s_ps,
                        qoh[:, j, :],
                        mx[:, j, :],
                        start=(j == 0),
                        stop=(j == 2 * BPC - 1),
                    )
                d_ps = pss.tile([BPC, B], F32)
                nc.tensor.matmul(d_ps, qsT, psT, start=True, stop=True)

                zl = small.tile([BPC, B], F32)
                nc.vector.tensor_scalar_mul(zl, s_ps, r50)
                zd = small.tile([BPC, B], F32)
                nc.vector.tensor_scalar_mul(zd, d_ps, ITAU)

                out_sb = small.tile([BPC, 3], F32)
                eps_ap = small.tile([BPC, 1], F32)
                nc.vector.memset(eps_ap, 1e-8)

                probs = []
                for col, z in ((0, zd), (1, zl)):
                    nmax = small.tile([BPC, 1], F32, name=f"nmax{col}")
                    nc.vector.reduce_max(out=nmax, in_=z, axis=AX.X, negate=True)
                    ez = small.tile([BPC, B], F32, name=f"ez{col}")
                    den = small.tile([BPC, 1], F32, name=f"den{col}")
                    nc.scalar.activation(
                        ez, z, ACT.Exp, bias=nmax, scale=1.0, accum_out=den
                    )
                    logz = small.tile([BPC, 1], F32, name=f"logz{col}")
                    nc.scalar.activation(logz, den, ACT.Ln, bias=zeros1)
                    rden = small.tile([BPC, 1], F32, name=f"rden{col}")
                    nc.vector.reciprocal(rden, den)
                    pr = small.tile([BPC, B], F32, name=f"pr{col}")
                    nc.vector.tensor_scalar_mul(pr, ez, rden)
                    probs.append(pr)
                    junk = small.tile([BPC, B], F32, name=f"junk{col}")
                    nc.vector.tensor_mul(junk, z, diag)
                    ztgt = small.tile([BPC, 1], F32, name=f"ztgt{col}")
                    nc.vector.reduce_sum(out=ztgt, in_=junk, axis=AX.X)
                    # -logp_tgt = logZ - ztgt - nmax   (nmax = -rowmax)
                    t1 = small.tile([BPC, 1], F32, name=f"t1{col}")
                    nc.vector.tensor_sub(t1, logz, ztgt)
                    nc.vector.tensor_sub(out_sb[:, col : col + 1], t1, nmax)

                dp, lp = probs
                ldp = small.tile([BPC, B], F32)
                nc.scalar.activation(ldp, dp, ACT.Ln, bias=eps_ap)
                llp = small.tile([BPC, B], F32)
                nc.scalar.activation(llp, lp, ACT.Ln, bias=eps_ap)
                dl = small.tile([BPC, B], F32)
                nc.vector.tensor_sub(dl, ldp, llp)
                junk_kl = small.tile([BPC, B], F32)
                nc.vector.tensor_mul(junk_kl, dp, dl)
                klrow = small.tile([BPC, 1], F32)
                nc.vector.reduce_sum(out=klrow, in_=junk_kl, axis=AX.X)
                nc.vector.tensor_copy(out_sb[:, 2:3], klrow)

                nc.sync.dma_start(out=out_d[:, :], in_=out_sb)

    nc.compile()
    return nc


_NC_CACHE = None


def _get_nc():
    global _NC_CACHE
    if _NC_CACHE is None:
        _NC_CACHE = _build_kernel()
    return _NC_CACHE


def _prep_in_maps(query_single, pos_single, query_multi, pos_multi, q_mask, p_mask):
    qs = np.ascontiguousarray(np.asarray(query_single, np.float32))
    ps = np.ascontiguousarray(np.asarray(pos_single, np.float32))
    qm = np.ascontiguousarray(np.asarray(query_multi, np.float32))
    pm = np.ascontiguousarray(np.asarray(pos_multi, np.float32))
    qmask = np.asarray(q_mask).astype(bool)
    pmask = np.asarray(p_mask).astype(bool)

    # Fold p_mask: overwrite invalid tokens with the doc's first valid token.
    # Duplicated sims never change the per-doc max.
    first_valid = pmask.argmax(axis=1)
    p_filled = pm.copy()
    for c in range(B):
        if not pmask[c].all():
            p_filled[c, ~pmask[c]] = pm[c, first_valid[c]]
    import ml_dtypes
    p3 = p_filled.reshape(B, 2, T // 2, D)
    p1T = np.ascontiguousarray(
        p3[:, 1].reshape(PCOLS // 2, D).T.astype(ml_dtypes.bfloat16)
    )
    pdT = np.ascontiguousarray(
        (p3[:, 0] - p3[:, 1]).reshape(PCOLS // 2, D).T.astype(ml_dtypes.bfloat16)
    )
    ident = np.eye(128, dtype=ml_dtypes.bfloat16)

    t_i = np.maximum(qmask.sum(axis=1), 1).astype(np.float32)
    psT = np.ascontiguousarray(ps.T)

    in_maps = []
    for c in range(NCORES):
        b0 = c * BPC
        qT = np.ascontiguousarray(
            qm[b0 : b0 + BPC].reshape(BPC * T, D).T.astype(ml_dtypes.bfloat16)
        )
        qsT = np.ascontiguousarray(qs[b0 : b0 + BPC].T)
        qoh = np.zeros((D, 2 * BPC, BPC), np.float32)
        for ib in range(BPC):
            for qc in range(2):
                qoh[:, ib * 2 + qc, ib] = qmask[b0 + ib, qc * 128 : (qc + 1) * 128]
        diag = np.zeros((BPC, B), np.float32)
        for ib in range(BPC):
            diag[ib, b0 + ib] = 1.0
        r50 = (ITAU / t_i[b0 : b0 + BPC]).reshape(BPC, 1).astype(np.float32)
        in_maps.append(
            {
                "p1T": p1T,
                "pdT": pdT,
                "identity": ident,
                "qT": qT,
                "qsT": qsT,
                "psT": psT,
                "qoh": qoh,
                "diag_oh": diag,
                "recip50t": r50,
            }
        )
    return in_maps


def run(inputs: dict, trace: bool = False):
    """Run the spmd kernel; returns (loss tuple, BassKernelResults)."""
    nc = _get_nc()
    in_maps = _prep_in_maps(**inputs)
    res = run_bass_kernel_spmd(
        nc, in_maps, core_ids=list(range(NCORES)), trace=trace
    )
    rows = np.concatenate([r["out"] for r in res.results], axis=0)  # [32, 3]
    single = rows[:, 0].mean(dtype=np.float64)
    multi = rows[:, 1].mean(dtype=np.float64)
    kl = rows[:, 2].mean(dtype=np.float64)
    total = single + multi + kl
    out = (
        np.float32(total),
        np.float32(single),
        np.float32(multi),
        np.float32(kl),
    )
    return out, res


def kernel(query_single, pos_single, query_multi, pos_multi, q_mask, p_mask):
    out, _ = run(
        {
            "query_single": query_single,
            "pos_single": pos_single,
            "query_multi": query_multi,
            "pos_multi": pos_multi,
            "q_mask": q_mask,
            "p_mask": p_mask,
        }
    )
    return out



# revision 8
# speedup vs baseline: 1.5458x; 1.5458x over previous
"""Trainium2 Bass kernel for nn_JinaPairTraining (dense CE + late-interaction
maxsim CE + KL between the two softmax distributions).

Sharding: data-parallel over queries. The 32 queries are binned 4-per-core so
that each core gets an equal number of *valid* 128-token query chunks
(q_mask-aware: a query with q_len <= 128 contributes one chunk, not two).
Every core holds the full pos side and computes its rows of both logit
matrices, the row-wise exp-sums, diagonal logits, and the KL cross term; the
host finishes with a handful of scalar logs and the mean.

Device-side structure (per core, J = chunks per core):
  * Q/P matmuls run in fp8e4m3 (inputs scaled x8) with DoubleRow packing
    (contraction 128 = 64 partitions x 2) -- 2x PE throughput.
  * p_mask is folded on the host (masked tokens replaced by the doc's first
    valid token -- duplicates never change a max).
  * Pair-max folded into PE+ACT: max(s0,s1) = s1 + relu(s0-s1); PE computes
    Q@(P0-P1) and Q@P1, ACT applies relu, an identity matmul accumulates.
  * The per-doc max reduce runs as a DVE tensor_tensor max tree (pairwise max
    consumes 2 inputs/cycle vs 1 for tensor_reduce) with a small final
    reduce_max.
  * q_mask + 1/t_i + 1/tau + fp8 scale correction are folded into the one-hot
    sum-over-q matmul operand.
  * Input DMAs are spread across the DVE/ACT/Pool/SP queues so their fixed
    overheads overlap; a tiny warm-up matmul at t~0 starts the PE ramp early.
"""

import math
import os
import sys

import numpy as np

for _p in ("/opt/trn_rl_repo",):
    if _p not in sys.path and os.path.isdir(_p):
        sys.path.insert(0, _p)

import ml_dtypes

import concourse.bacc as bacc
import concourse.tile as tile
from concourse import mybir
from concourse.bass_utils import run_bass_kernel_spmd

B, T, D = 32, 256, 128
TAU = 0.02
ITAU = 1.0 / TAU
NCORES = 8
ROWS = B // NCORES          # 4 query rows per core
PCOLS = B * T               # 8192 pos token columns
FCOLS = PCOLS // 2          # 4096 folded (pair-max) columns
NREG = 4                    # folded regions of 1024 cols (2 PSUM banks) each
REG = FCOLS // NREG         # 1024
SCALE = 8.0                 # fp8 pre-scale; sims come out x64
SIM_SCALE = SCALE * SCALE
ZBIAS = -25.0               # safe exp shift: |z| <= 50 for cosine logits

F32 = mybir.dt.float32
BF16 = mybir.dt.bfloat16
FP8 = mybir.dt.float8e4
AX = mybir.AxisListType
ALU = mybir.AluOpType
ACT = mybir.ActivationFunctionType
PM = mybir.MatmulPerfMode
NPF8 = ml_dtypes.float8_e4m3

# smalls layout (f32 [128, 160]): qoh cols 0:4J, qsT 32:36, psT 36:68,
# diag2 (partitions 0:ROWS) 68:132
SM_QS = 32
SM_PS = 36
SM_DG = 68
SM_W = 160


def _to_dr(a):
    """[X, D] (row-major, D=128 contraction) -> DoubleRow layout [64, 2, X]."""
    return np.ascontiguousarray(a.T.reshape(2, 64, -1).transpose(1, 0, 2))


def _build_kernel(J):
    nc = bacc.Bacc(None, target_bir_lowering=False, debug=False)

    p8_d = nc.dram_tensor("p8", [64, 2, 2, FCOLS], FP8, kind="ExternalInput")
    q8_d = nc.dram_tensor("q8", [64, 2, J * 128], FP8, kind="ExternalInput")
    ident_d = nc.dram_tensor("ident", [128, 128], BF16, kind="ExternalInput")
    smalls_d = nc.dram_tensor("smalls", [128, SM_W], F32, kind="ExternalInput")
    out_d = nc.dram_tensor("out", [ROWS, 5], F32, kind="ExternalOutput")

    with tile.TileContext(nc) as tc:
        with (
            tc.tile_pool(name="sb", bufs=1) as sb,
            tc.tile_pool(name="rp", bufs=3) as rp,
        ):
            p8 = sb.tile([64, 2, 2, FCOLS], FP8, name="p8")
            q8 = sb.tile([64, 2, J * 128], FP8, name="q8")
            ident = sb.tile([128, 128], BF16, name="ident")
            smalls = sb.tile([128, SM_W], F32, name="smalls")
            mx = sb.tile([128, J, B], F32, name="mx")
            warm_a = sb.tile([128, 2], BF16, name="warm_a")
            warm_f = sb.tile([1, 2], F32, name="warm_f")

            # PE ramp + ACT exp-table warm-up at t~0 (1 PSUM bank, freed
            # before the main pools open).
            nc.vector.memset(warm_a, 0.0)
            with tc.tile_pool(name="wps", bufs=1, space="PSUM") as wpsp:
                wps = wpsp.tile([1, 2], F32, name="wps")
                nc.tensor.matmul(
                    wps, warm_a[:, 0:1], warm_a, start=True, stop=True
                )
                nc.scalar.activation(warm_f, wps, ACT.Exp)
                nc.scalar.activation(warm_f, warm_f, ACT.Relu)

            # Input DMAs on separate queues so fixed overheads overlap.
            # smalls is tail-only data, so it rides second on the Pool queue.
            nc.scalar.dma_start(out=q8, in_=q8_d[:, :, :])
            nc.gpsimd.dma_start(out=ident, in_=ident_d[:, :])
            nc.gpsimd.dma_start(out=smalls, in_=smalls_d[:, :])
            for r in range(NREG):
                nc.sync.dma_start(
                    out=p8[:, :, :, r * REG : (r + 1) * REG],
                    in_=p8_d[:, :, :, r * REG : (r + 1) * REG],
                )

            with (
                tc.tile_pool(name="pd", bufs=2, space="PSUM") as pdp,
                tc.tile_pool(name="pm", bufs=2, space="PSUM") as pmp,
            ):
                for j in range(J):
                    qj = q8[:, :, j * 128 : (j + 1) * 128]
                    for r in range(NREG):
                        ps_d = pdp.tile([128, REG], F32, name="ps_d")
                        ps_m = pmp.tile([128, REG], F32, name="ps_m")
                        for h in range(2):
                            cs = slice(r * REG + h * 512, r * REG + (h + 1) * 512)
                            nc.tensor.matmul(
                                ps_d[:, h * 512 : (h + 1) * 512],
                                qj,
                                p8[:, :, 1, cs],
                                start=True,
                                stop=True,
                                perf_mode=PM.DoubleRow,
                            )
                        for h in range(2):
                            cs = slice(r * REG + h * 512, r * REG + (h + 1) * 512)
                            nc.tensor.matmul(
                                ps_m[:, h * 512 : (h + 1) * 512],
                                qj,
                                p8[:, :, 0, cs],
                                start=True,
                                stop=False,
                                perf_mode=PM.DoubleRow,
                            )
                        relu = rp.tile([128, REG], BF16, name="relu")
                        nc.scalar.activation(relu, ps_d, ACT.Relu)
                        for h in range(2):
                            nc.tensor.matmul(
                                ps_m[:, h * 512 : (h + 1) * 512],
                                ident,
                                relu[:, h * 512 : (h + 1) * 512],
                                start=False,
                                stop=True,
                            )
                        # segmented max over each doc's 128 folded values
                        nc.vector.reduce_max(
                            out=mx[:, j, r * 8 : (r + 1) * 8],
                            in_=ps_m.rearrange("p (g s) -> p g s", s=T // 2),
                            axis=AX.X,
                        )

            # tail: logits rows, exp-sums, diag logits, KL cross term
            with tc.tile_pool(name="zp", bufs=1, space="PSUM") as zp:
                z = zp.tile([ROWS, 2 * B], F32, name="z")
                nc.tensor.matmul(
                    z[:, 0:B],
                    smalls[:, SM_QS : SM_QS + ROWS],
                    smalls[:, SM_PS : SM_PS + B],
                    start=True,
                    stop=True,
                )
                for j in range(J):
                    nc.tensor.matmul(
                        z[:, B : 2 * B],
                        smalls[:, j * ROWS : (j + 1) * ROWS],
                        mx[:, j, :],
                        start=(j == 0),
                        stop=(j == J - 1),
                    )

                ez = sb.tile([ROWS, 2 * B], F32, name="ez")
                zbias = sb.tile([ROWS, 1], F32, name="zbias")
                nc.vector.memset(zbias, ZBIAS)
                nc.scalar.activation(ez, z, ACT.Exp, bias=zbias)
                out_sb = sb.tile([ROWS, 5], F32, name="out_sb")
                nc.vector.reduce_sum(
                    out=out_sb[:, 0:2],
                    in_=ez.rearrange("p (g s) -> p g s", s=B),
                    axis=AX.X,
                )
                junk = sb.tile([ROWS, 2 * B], F32, name="junk")
                nc.vector.tensor_mul(
                    junk, z, smalls[0:ROWS, SM_DG : SM_DG + 2 * B]
                )
                nc.vector.reduce_sum(
                    out=out_sb[:, 2:4],
                    in_=junk.rearrange("p (g s) -> p g s", s=B),
                    axis=AX.X,
                )
                zsb = sb.tile([ROWS, 2 * B], F32, name="zsb")
                nc.vector.tensor_copy(zsb, z)
                zdiff = sb.tile([ROWS, B], F32, name="zdiff")
                nc.vector.tensor_sub(zdiff, zsb[:, 0:B], zsb[:, B : 2 * B])
                w = sb.tile([ROWS, B], F32, name="w")
                nc.vector.tensor_mul(w, ez[:, 0:B], zdiff)
                nc.vector.reduce_sum(out=out_sb[:, 4:5], in_=w, axis=AX.X)

                nc.sync.dma_start(out=out_d[:, :], in_=out_sb)

    nc.compile()
    return nc


_NC_CACHE = {}
_NC_LAST = None


def _get_nc(J=None):
    global _NC_LAST
    if J is None:
        assert _NC_LAST is not None
        return _NC_LAST
    if J not in _NC_CACHE:
        _NC_CACHE[J] = _build_kernel(J)
    _NC_LAST = _NC_CACHE[J]
    return _NC_LAST


def _plan_cores(qmask):
    """Bin the 32 queries 4-per-core, balancing total 128-token chunks."""
    q_len = qmask.sum(axis=1).astype(np.int64)
    chunks = np.maximum(1, -(-q_len // 128))  # ceil, min 1
    order = np.argsort(-chunks, kind="stable")
    bins = [[] for _ in range(NCORES)]
    sums = [0] * NCORES
    for b in order:
        cand = min(
            (i for i in range(NCORES) if len(bins[i]) < ROWS),
            key=lambda i: (sums[i], len(bins[i])),
        )
        bins[cand].append(int(b))
        sums[cand] += int(chunks[b])
    J = max(sums)
    return bins, chunks, J


def _prep_in_maps(query_single, pos_single, query_multi, pos_multi, q_mask, p_mask):
    qs = np.asarray(query_single, np.float32)
    ps = np.asarray(pos_single, np.float32)
    qm = np.asarray(query_multi, np.float32)
    pm = np.asarray(pos_multi, np.float32)
    qmask = np.asarray(q_mask).astype(bool)
    pmask = np.asarray(p_mask).astype(bool)

    # Fold p_mask: masked tokens replaced by the doc's first valid token.
    first_valid = pmask.argmax(axis=1)
    p_filled = pm.copy()
    for c in range(B):
        if not pmask[c].all():
            p_filled[c, ~pmask[c]] = pm[c, first_valid[c]]

    # P in DoubleRow fp8: [64, 2, {P1, Pd}, 4096]
    p3 = p_filled.reshape(B, 2, T // 2, D)
    p1 = np.ascontiguousarray(p3[:, 1]).reshape(FCOLS, D)
    pd = np.ascontiguousarray(p3[:, 0] - p3[:, 1]).reshape(FCOLS, D)
    p8 = np.stack(
        [_to_dr(p1 * SCALE), _to_dr(pd * SCALE)], axis=2
    ).astype(NPF8)
    p8 = np.ascontiguousarray(p8)

    ident = np.ascontiguousarray(np.eye(128, dtype=ml_dtypes.bfloat16))
    t_i = np.maximum(qmask.sum(axis=1), 1).astype(np.float64)

    bins, chunks, J = _plan_cores(qmask)

    in_maps = []
    for i in range(NCORES):
        slots = [(b, c) for b in bins[i] for c in range(int(chunks[b]))]
        assert len(slots) <= J
        q8 = np.zeros((64, 2, J * 128), NPF8)
        smalls = np.zeros((128, SM_W), np.float32)
        for j, (b, c) in enumerate(slots):
            row = bins[i].index(b)
            blk = qm[b, c * 128 : (c + 1) * 128, :]  # [128 tok, 128 d]
            q8[:, :, j * 128 : (j + 1) * 128] = _to_dr(blk * SCALE).astype(NPF8)
            smalls[:, j * ROWS + row] = qmask[b, c * 128 : (c + 1) * 128] * (
                ITAU / (SIM_SCALE * t_i[b])
            )
        smalls[:, SM_QS : SM_QS + ROWS] = qs[bins[i]].T * ITAU
        smalls[:, SM_PS : SM_PS + B] = ps.T
        for row, b in enumerate(bins[i]):
            smalls[row, SM_DG + b] = 1.0
            smalls[row, SM_DG + B + b] = 1.0
        in_maps.append(
            {"p8": p8, "q8": q8, "ident": ident, "smalls": smalls}
        )
    return in_maps, J


def run(inputs: dict, trace: bool = False):
    """Run the spmd kernel; returns (loss tuple, BassKernelResults)."""
    in_maps, J = _prep_in_maps(**inputs)
    nc = _get_nc(J)
    res = run_bass_kernel_spmd(
        nc, in_maps, core_ids=list(range(NCORES)), trace=trace
    )
    rows = np.concatenate([r["out"] for r in res.results], axis=0).astype(
        np.float64
    )  # [32, 5] = den_d, den_l, ztgt_d, ztgt_l, skl
    den_d, den_l, ztd, ztl, skl = rows.T
    sl = (-ZBIAS) + np.log(den_d) - ztd
    ml = (-ZBIAS) + np.log(den_l) - ztl
    kl = skl / den_d - np.log(den_d) + np.log(den_l)
    single = sl.mean()
    multi = ml.mean()
    klm = kl.mean()
    total = single + multi + klm
    out = (
        np.float32(total),
        np.float32(single),
        np.float32(multi),
        np.float32(klm),
    )
    return out, res


def kernel(query_single, pos_single, query_multi, pos_multi, q_mask, p_mask):
    out, _ = run(
        {
            "query_single": query_single,
            "pos_single": pos_single,
            "query_multi": query_multi,
            "pos_multi": pos_multi,
            "q_mask": q_mask,
            "p_mask": p_mask,
        }
    )
    return out


# revision 44
# speedup vs baseline: 1.9759x; 1.2782x over previous
"""Trainium2 Bass kernel for nn_JinaPairTraining (dense CE + late-interaction
maxsim CE + KL between the two softmax distributions).

Sharding: data-parallel over queries. The 32 queries are binned 4-per-core so
that each core gets an equal number of *valid* 128-token query chunks
(q_mask-aware: a query with q_len <= 128 contributes one chunk, not two).
Every core holds the full pos side and computes its rows of both logit
matrices, the row-wise exp-sums, diagonal logits, and the KL cross term; the
host finishes with a handful of scalar logs and the mean.

Device-side structure (per core, J = chunks per core):
  * Q/P matmuls run in fp8e4m3 (inputs scaled x8) with DoubleRow packing
    (contraction 128 = 64 partitions x 2) -- 2x PE throughput.
  * p_mask is folded on the host (masked tokens replaced by the doc's first
    valid token -- duplicates never change a max).
  * Pair-max folded into PE+ACT: max(s0,s1) = s1 + relu(s0-s1); PE computes
    Q@(P0-P1) and Q@P1, ACT applies relu, an identity matmul accumulates.
  * The per-doc max reduce runs as a DVE tensor_tensor max tree (pairwise max
    consumes 2 inputs/cycle vs 1 for tensor_reduce) with a small final
    reduce_max.
  * q_mask + 1/t_i + 1/tau + fp8 scale correction are folded into the one-hot
    sum-over-q matmul operand.
  * Input DMAs are spread across the DVE/ACT/Pool/SP queues so their fixed
    overheads overlap; a tiny warm-up matmul at t~0 starts the PE ramp early.
"""

import math
import os
import sys

import numpy as np

for _p in ("/opt/trn_rl_repo",):
    if _p not in sys.path and os.path.isdir(_p):
        sys.path.insert(0, _p)

import ml_dtypes

import concourse.bacc as bacc
import concourse.tile as tile
from concourse import mybir
from concourse.bass_utils import run_bass_kernel_spmd

B, T, D = 32, 256, 128
TAU = 0.02
ITAU = 1.0 / TAU
NCORES = 8
ROWS = B // NCORES          # 4 query rows per core
PCOLS = B * T               # 8192 pos token columns
FCOLS = PCOLS // 2          # 4096 folded (pair-max) columns
NREG = 4                    # folded regions of 1024 cols (2 PSUM banks) each
REG = FCOLS // NREG         # 1024
SCALE = 8.0                 # fp8 pre-scale; sims come out x64
SIM_SCALE = SCALE * SCALE
ZBIAS = -25.0               # safe exp shift: |z| <= 50 for cosine logits

F32 = mybir.dt.float32
BF16 = mybir.dt.bfloat16
FP8 = mybir.dt.float8e4
AX = mybir.AxisListType
ALU = mybir.AluOpType
ACT = mybir.ActivationFunctionType
PM = mybir.MatmulPerfMode
NPF8 = ml_dtypes.float8_e4m3

# smalls layout (f32 [128, 160]): qoh cols 0:4J, qsT 32:36, psT 36:68,
# diag2 (partitions 0:ROWS) 68:132
SM_QS = 32
SM_PS = 36
SM_DG = 68
SM_W = 160


def _to_dr(a):
    """[X, D] (row-major, D=128 contraction) -> DoubleRow layout [64, 2, X]."""
    return np.ascontiguousarray(a.T.reshape(2, 64, -1).transpose(1, 0, 2))


def _split512(size):
    """Split [0, size) into matmul-sized pieces of <= 512 columns."""
    cuts = []
    o = 0
    while o < size:
        w = min(512, size - o)
        cuts.append((o, w))
        o += w
    return cuts


def _build_kernel(J, plan):
    """plan: tuple of single-seg chunks (seg, count) over folded columns."""
    chunk_sizes = [s * c for s, c in plan]
    fcols = sum(chunk_sizes)
    # Pair chunks so one ps_d tile + one relu covers both, but only when the
    # second matmul's PSUM output stays bank-legal: first chunk exactly 512
    # (second starts at the bank boundary) or both fit in one 512 bank.
    pairs = [(i, None) for i in range(len(plan))]

    nc = bacc.Bacc(None, target_bir_lowering=False, debug=False)

    p8_d = nc.dram_tensor("p8", [64, 2, 2, fcols], FP8, kind="ExternalInput")
    q8_d = nc.dram_tensor("q8", [64, 2, J * 128], FP8, kind="ExternalInput")
    ident_d = nc.dram_tensor("ident", [128, 128], BF16, kind="ExternalInput")
    smalls_d = nc.dram_tensor("smalls", [128, SM_W], F32, kind="ExternalInput")
    out_d = nc.dram_tensor("out", [ROWS, 6], F32, kind="ExternalOutput")

    with tile.TileContext(nc) as tc:
        with (
            tc.tile_pool(name="sb", bufs=1) as sb,
            tc.tile_pool(name="rp", bufs=3) as rp,
        ):
            p8 = sb.tile([64, 2, 2, fcols], FP8, name="p8")
            q8 = sb.tile([64, 2, J * 128], FP8, name="q8")
            ident = sb.tile([128, 128], BF16, name="ident")
            smalls = sb.tile([128, SM_W], F32, name="smalls")
            mx = sb.tile([128, J, B], F32, name="mx")
            warm_a = sb.tile([128, 2], BF16, name="warm_a")
            warm_f = sb.tile([1, 2], F32, name="warm_f")

            # PE ramp + ACT exp-table warm-up at t~0 (1 PSUM bank, freed
            # before the main pools open).
            nc.vector.memset(warm_a, 0.0)
            with tc.tile_pool(name="wps", bufs=1, space="PSUM") as wpsp:
                wps = wpsp.tile([1, 2], F32, name="wps")
                nc.tensor.matmul(
                    wps, warm_a[:, 0:1], warm_a, start=True, stop=True
                )
                nc.scalar.activation(warm_f, wps, ACT.Exp)
                nc.scalar.activation(warm_f, warm_f, ACT.Relu)

            # Input DMAs: p regions own the HWDGE path (region 0 gates the
            # first matmuls); q8/ident/smalls ride the Pool SWDGE queue in
            # need-order (smalls is tail-only data).
            nc.scalar.dma_start(out=q8, in_=q8_d[:, :, :])
            nc.gpsimd.dma_start(out=ident, in_=ident_d[:, :])
            nc.gpsimd.dma_start(out=smalls, in_=smalls_d[:, :])
            # p DMAs chunk-aligned, ~4 chunks (<=2048 folded cols) each
            chunk_off = [0]
            for cs in chunk_sizes:
                chunk_off.append(chunk_off[-1] + cs)
            dma_cuts = sorted(
                {chunk_off[0], chunk_off[1]}
                | {chunk_off[i] for i in range(1, len(chunk_sizes), 4)}
                | {fcols}
            )
            for a, b in zip(dma_cuts[:-1], dma_cuts[1:]):
                nc.sync.dma_start(
                    out=p8[:, :, :, a:b], in_=p8_d[:, :, :, a:b]
                )

            # doc-position offset of each chunk in sorted-doc order
            dpos_off = [0]
            for _, c in plan:
                dpos_off.append(dpos_off[-1] + c)

            ez = sb.tile([ROWS, 2 * B], F32, name="ez")
            zbias = sb.tile([ROWS, 1], F32, name="zbias")
            nc.vector.memset(zbias, ZBIAS)
            out_sb = sb.tile([ROWS, 6], F32, name="out_sb")
            junk = sb.tile([ROWS, 2 * B], F32, name="junk")
            wk = sb.tile([ROWS, 2 * B], F32, name="wk")

            with (
                tc.tile_pool(name="zp", bufs=1, space="PSUM") as zp,
                tc.tile_pool(name="pd", bufs=4, space="PSUM") as pdp,
                tc.tile_pool(name="pm", bufs=3, space="PSUM") as pmp,
            ):
                # dense-logit side runs entirely during the prologue/main
                z = zp.tile([ROWS, 2 * B], F32, name="z")
                nc.tensor.matmul(
                    z[:, 0:B],
                    smalls[:, SM_QS : SM_QS + ROWS],
                    smalls[:, SM_PS : SM_PS + B],
                    start=True,
                    stop=True,
                )
                nc.scalar.activation(ez[:, 0:B], z[:, 0:B], ACT.Exp, bias=zbias)
                nc.vector.reduce_sum(out=out_sb[:, 0:1], in_=ez[:, 0:B], axis=AX.X)
                nc.vector.scalar_tensor_tensor(
                    out=junk[:, 0:B], in0=z[:, 0:B], scalar=1.0,
                    in1=smalls[0:ROWS, SM_DG : SM_DG + B],
                    op0=ALU.mult, op1=ALU.mult, accum_out=out_sb[:, 2:3],
                )
                nc.vector.scalar_tensor_tensor(
                    out=wk[:, 0:B], in0=z[:, 0:B], scalar=1.0, in1=ez[:, 0:B],
                    op0=ALU.mult, op1=ALU.mult, accum_out=out_sb[:, 4:5],
                )

                for j in range(J):
                    qj = q8[:, :, j * 128 : (j + 1) * 128]
                    for ca, cb in pairs:
                        sa = chunk_sizes[ca]
                        sb_ = chunk_sizes[cb] if cb is not None else 0
                        # diff matmuls for the pair share one tile + one relu
                        ps_d = pdp.tile([128, 512], F32, name="ps_d")
                        nc.tensor.matmul(
                            ps_d[:, 0:sa],
                            qj,
                            p8[:, :, 1, chunk_off[ca] : chunk_off[ca] + sa],
                            start=True,
                            stop=True,
                            perf_mode=PM.DoubleRow,
                        )
                        if cb is not None:
                            nc.tensor.matmul(
                                ps_d[:, sa : sa + sb_],
                                qj,
                                p8[:, :, 1, chunk_off[cb] : chunk_off[cb] + sb_],
                                start=True,
                                stop=True,
                                perf_mode=PM.DoubleRow,
                            )
                        relu = rp.tile([128, 512], BF16, name="relu")
                        nc.scalar.activation(
                            relu[:, 0 : sa + sb_], ps_d[:, 0 : sa + sb_], ACT.Relu
                        )
                        for ci, off in ((ca, 0), (cb, sa)):
                            if ci is None:
                                continue
                            seg, cnt = plan[ci]
                            size = chunk_sizes[ci]
                            base = chunk_off[ci]
                            ps_m = pmp.tile([128, 512], F32, name="ps_m")
                            nc.tensor.matmul(
                                ps_m[:, 0:size],
                                qj,
                                p8[:, :, 0, base : base + size],
                                start=True,
                                stop=False,
                                perf_mode=PM.DoubleRow,
                            )
                            nc.tensor.matmul(
                                ps_m[:, 0:size],
                                ident,
                                relu[:, off : off + size],
                                start=False,
                                stop=True,
                            )
                            dpos = dpos_off[ci]
                            nc.vector.reduce_max(
                                out=mx[:, j, dpos : dpos + cnt],
                                in_=ps_m[:, 0:size].rearrange(
                                    "p (g s) -> p g s", s=seg
                                ),
                                axis=AX.X,
                            )
                    # late-logit accumulation as soon as mx[:, j] is complete
                    nc.tensor.matmul(
                        z[:, B : 2 * B],
                        smalls[:, j * ROWS : (j + 1) * ROWS],
                        mx[:, j, :],
                        start=(j == 0),
                        stop=(j == J - 1),
                    )

                # late tail: exp-sum, diag logit, KL cross term
                nc.scalar.activation(
                    ez[:, B : 2 * B], z[:, B : 2 * B], ACT.Exp, bias=zbias
                )
                nc.vector.reduce_sum(
                    out=out_sb[:, 1:2], in_=ez[:, B : 2 * B], axis=AX.X
                )
                nc.vector.scalar_tensor_tensor(
                    out=junk[:, B : 2 * B], in0=z[:, B : 2 * B], scalar=1.0,
                    in1=smalls[0:ROWS, SM_DG + B : SM_DG + 2 * B],
                    op0=ALU.mult, op1=ALU.mult, accum_out=out_sb[:, 3:4],
                )
                nc.vector.scalar_tensor_tensor(
                    out=wk[:, B : 2 * B], in0=z[:, B : 2 * B], scalar=1.0,
                    in1=ez[:, 0:B],
                    op0=ALU.mult, op1=ALU.mult, accum_out=out_sb[:, 5:6],
                )

                nc.sync.dma_start(out=out_d[:, :], in_=out_sb)

    nc.compile()
    return nc


_NC_CACHE = {}
_NC_LAST = None


def _get_nc(key=None):
    global _NC_LAST
    if key is None:
        assert _NC_LAST is not None
        return _NC_LAST
    if key not in _NC_CACHE:
        _NC_CACHE[key] = _build_kernel(*key)
    _NC_LAST = _NC_CACHE[key]
    return _NC_LAST


def _plan_p(pmask):
    """Doc-compaction plan: pad each doc's valid prefix to a multiple of 64
    tokens (>= 128), sort docs by padded size (desc), split each size class
    into single-seg chunks of <= 512 folded cols. Returns (doc order, per-doc
    folded seg, chunk plan); plan entries are (seg, count)."""
    p_len = np.maximum(pmask.sum(axis=1).astype(np.int64), 1)
    pad = np.clip(-(-p_len // 64) * 64, 128, T)
    order = np.argsort(-pad, kind="stable")
    segs = (pad[order] // 2).astype(int)
    plan = []
    i = 0
    while i < len(segs):
        s = int(segs[i])
        n_class = 1
        while i + n_class < len(segs) and segs[i + n_class] == s:
            n_class += 1
        cap = 512 // s
        left = n_class
        while left > 0:
            take = min(cap, left)
            plan.append((s, take))
            left -= take
        i += n_class
    # smallest chunk first: first p DMA (and first compute) needs less data,
    # and doc order must follow chunk order for the reduce/dpos mapping
    chunk_order = sorted(range(len(plan)), key=lambda k: plan[k][0] * plan[k][1])
    new_order = []
    pos = np.concatenate([[0], np.cumsum([c for _, c in plan])]).astype(int)
    for k in chunk_order:
        new_order.extend(order[pos[k] : pos[k + 1]])
    order = np.array(new_order)
    plan = tuple(plan[k] for k in chunk_order)
    segs = (np.array([s for s, c in plan for _ in range(c)])).astype(int)
    return order, segs, tuple(plan)


def _plan_cores(qmask):
    """Bin the 32 queries 4-per-core, balancing total *valid* q tokens.
    Valid tokens from a core's queries are packed densely into 128-lane
    chunks (lanes may mix queries -- the one-hot sum matmul untangles)."""
    q_len = np.maximum(qmask.sum(axis=1).astype(np.int64), 1)
    order = np.argsort(-q_len, kind="stable")
    bins = [[] for _ in range(NCORES)]
    sums = [0] * NCORES
    for b in order:
        cand = min(
            (i for i in range(NCORES) if len(bins[i]) < ROWS),
            key=lambda i: (sums[i], len(bins[i])),
        )
        bins[cand].append(int(b))
        sums[cand] += int(q_len[b])
    J = max(-(-s // 128) for s in sums)
    return bins, q_len, J


def _prep_in_maps(query_single, pos_single, query_multi, pos_multi, q_mask, p_mask):
    qs = np.asarray(query_single, np.float32)
    ps = np.asarray(pos_single, np.float32)
    qm = np.asarray(query_multi, np.float32)
    pm = np.asarray(pos_multi, np.float32)
    qmask = np.asarray(q_mask).astype(bool)
    pmask = np.asarray(p_mask).astype(bool)

    # Fold p_mask: masked tokens replaced by the doc's first valid token.
    first_valid = pmask.argmax(axis=1)
    p_filled = pm.copy()
    for c in range(B):
        if not pmask[c].all():
            p_filled[c, ~pmask[c]] = pm[c, first_valid[c]]

    # Doc compaction + pair fold. For doc d padded to 2*seg tokens, column
    # block = [P1 = tokens seg:2seg | Pd = tokens 0:seg - P1], in sorted-doc
    # order. P in DoubleRow fp8: [64, 2, {P1, Pd}, fcols].
    p_order, p_segs, plan = _plan_p(pmask)
    fcols = int(p_segs.sum())
    p1 = np.empty((fcols, D), np.float32)
    pd = np.empty((fcols, D), np.float32)
    o = 0
    for d, seg in zip(p_order, p_segs):
        hi = p_filled[d, seg : 2 * seg]
        lo = p_filled[d, 0:seg]
        p1[o : o + seg] = hi
        pd[o : o + seg] = lo - hi
        o += seg
    p8 = np.stack(
        [_to_dr(p1 * SCALE), _to_dr(pd * SCALE)], axis=2
    ).astype(NPF8)
    p8 = np.ascontiguousarray(p8)
    p_pos = np.empty(B, np.int64)  # doc -> column position in sorted order
    p_pos[p_order] = np.arange(B)

    ident = np.ascontiguousarray(np.eye(128, dtype=ml_dtypes.bfloat16))
    t_i = np.maximum(qmask.sum(axis=1), 1).astype(np.float64)

    bins, q_len, J = _plan_cores(qmask)

    in_maps = []
    for i in range(NCORES):
        # densely pack the bin's valid q tokens into J 128-lane chunks
        toks = np.concatenate(
            [
                np.stack(
                    [np.full(q_len[b], row), np.nonzero(qmask[b])[0]], axis=1
                )
                for row, b in enumerate(bins[i])
            ]
        )  # [n_tok, (row, t)]
        assert len(toks) <= J * 128
        qcat = np.zeros((J * 128, D), np.float32)
        qcat[: len(toks)] = qm[np.array(bins[i])[toks[:, 0]], toks[:, 1], :]
        q8 = _to_dr(qcat * SCALE).astype(NPF8)
        smalls = np.zeros((128, SM_W), np.float32)
        for j in range(J):
            for lane, (row, _t) in enumerate(toks[j * 128 : (j + 1) * 128]):
                b = bins[i][row]
                smalls[lane, j * ROWS + row] = ITAU / (SIM_SCALE * t_i[b])
        smalls[:, SM_QS : SM_QS + ROWS] = qs[bins[i]].T * ITAU
        smalls[:, SM_PS : SM_PS + B] = ps[p_order].T
        for row, b in enumerate(bins[i]):
            smalls[row, SM_DG + p_pos[b]] = 1.0
            smalls[row, SM_DG + B + p_pos[b]] = 1.0
        in_maps.append(
            {"p8": p8, "q8": q8, "ident": ident, "smalls": smalls}
        )
    return in_maps, (J, plan)


def run(inputs: dict, trace: bool = False):
    """Run the spmd kernel; returns (loss tuple, BassKernelResults)."""
    in_maps, key = _prep_in_maps(**inputs)
    nc = _get_nc(key)
    res = run_bass_kernel_spmd(
        nc, in_maps, core_ids=list(range(NCORES)), trace=trace
    )
    rows = np.concatenate([r["out"] for r in res.results], axis=0).astype(
        np.float64
    )  # [32, 6] = den_d, den_l, ztgt_d, ztgt_l, skl_a, skl_b
    den_d, den_l, ztd, ztl, skl_a, skl_b = rows.T
    sl = (-ZBIAS) + np.log(den_d) - ztd
    ml = (-ZBIAS) + np.log(den_l) - ztl
    kl = (skl_a - skl_b) / den_d - np.log(den_d) + np.log(den_l)
    single = sl.mean()
    multi = ml.mean()
    klm = kl.mean()
    total = single + multi + klm
    out = (
        np.float32(total),
        np.float32(single),
        np.float32(multi),
        np.float32(klm),
    )
    return out, res


def kernel(query_single, pos_single, query_multi, pos_multi, q_mask, p_mask):
    out, _ = run(
        {
            "query_single": query_single,
            "pos_single": pos_single,
            "query_multi": query_multi,
            "pos_multi": pos_multi,
            "q_mask": q_mask,
            "p_mask": p_mask,
        }
    )
    return out
